# revision 31
# baseline (speedup 1.0000x reference)
"""AFT-local attention on 8 Trainium2 NeuronCores.

Reference (per batch element b, S=2048, D=512, window=128):
    query = q @ Wq.T + bq;  Q_ = sigmoid(query)
    key_p = k @ Wk.T + bk;  ek = exp(key_p)
    value = v @ Wv.T + bv;  ekv = ek * value
    ew    = exp(w_bias * local_mask)          # S x S, == 1 outside the band
    num_raw = ew @ ekv ; den = ew @ ek        # dense S x S einsums
    num  = Q_ * num_raw;  x = num / den
    out1 = x @ out_w.T + out_b
    return (out1, num)

Key decomposition: ew = 1 + (exp(wb_masked) - 1) restricted to the band
|i-j| < 128, so  ew @ Y = colsum(Y) + EWM1_band @ Y  where the banded part
only touches <=3 column tiles of 128 per row tile of 128 (46 block matmuls
instead of 256 dense ones).  colsum(Y) is one ones-vector matmul per
sequence tile, and its broadcast back to all 128 output partitions is a
K=1 matmul accumulated into the same PSUM tile.

Sharding: pure data-parallel; batch B=8 -> one batch element per core.

Implementation notes (all measured on silicon):
- Matmuls run as float32r (single-pass fp32 on the 128x128 PE array, 1
  cycle/row at free-dim 512 vs 4 for plain fp32; ~1.5e-4 rel err).
- The S x S column sums are a DVE accumulation chain + one ones-vector PE
  matmul; the per-row-tile broadcast back over 128 partitions is a K=1
  matmul accumulated straight into the band PSUM group.
- 1/den uses reciprocal_approx_fast (~5x faster than DVE RECIPROCAL at
  ~4e-6 rel err); x = num * rcp.
- Phase C is software-pipelined: the PE transpose + output projection of
  row tile i-1 are emitted after the band matmuls of tile i so the PE
  never stalls on the DVE epilogue.
- All DMA sources are host-packed so every partition line is one
  contiguous burst; the band is shipped as bf16 (its values are tiny
  biases; error contribution ~1e-5) and weights/first k-tiles are split
  across several DMA queues to cut startup latency.
- Final 1/8-microsecond-scale schedule: ~120us of PE work at 92-99%
  occupancy inside its window; HW exec ~155-190us/8 cores depending on
  chip load (vs ~283us for the first working version).
"""

import sys

if "/opt/trn_rl_repo" not in sys.path:
    sys.path.insert(0, "/opt/trn_rl_repo")

import numpy as np

import concourse.bacc as bacc
import concourse.mybir as mybir
import concourse.tile as tile
from concourse.bass import ts

from concourse.bass_utils import run_bass_kernel_spmd
from concourse.masks import make_identity

F32 = mybir.dt.float32
F32R = mybir.dt.float32r
EXP = mybir.ActivationFunctionType.Exp
SIGMOID = mybir.ActivationFunctionType.Sigmoid
BF16 = mybir.dt.bfloat16

# Optional: bf16 for q/k/v + their projection weights halves the dominant
# DMA stream for ~6% speedup, but exp(key) amplifies bf16 k-rounding to
# ~1e-2 absmax error on `num` (vs 4.6e-4 in full f32).  Kept OFF: accuracy
# margin is worth more than the 6%.
USE_BF16_INPUTS = False
IN_DT = BF16 if USE_BF16_INPUTS else F32
IN_MM_DT = BF16 if USE_BF16_INPUTS else F32R

S = 2048
D = 512
P = 128
NT = S // P  # 16 sequence tiles
NC = D // P  # 4 contraction chunks of the model dim
N_CORES = 8


def _band_blocks(i):
    """Valid (jl, j) column-tile neighbors for row tile i."""
    return [(jl, i - 1 + jl) for jl in range(3) if 0 <= i - 1 + jl < NT]


def build(with_biases):
    nc = bacc.Bacc(None, target_bir_lowering=False, debug=False)

    # host-packed layouts: every [128, ...] DMA partition line is contiguous
    qP_d = nc.dram_tensor("qP", [NT, P, NC * P], IN_DT, kind="ExternalInput")
    kP_d = nc.dram_tensor("kP", [NT, P, NC * P], IN_DT, kind="ExternalInput")
    vP_d = nc.dram_tensor("vP", [NT, P, NC * P], IN_DT, kind="ExternalInput")
    wq_d = nc.dram_tensor("wqP", [P, NC * D], IN_DT, kind="ExternalInput")
    wk_d = nc.dram_tensor("wkP", [P, NC * D], IN_DT, kind="ExternalInput")
    wv_d = nc.dram_tensor("wvP", [P, NC * D], IN_DT, kind="ExternalInput")
    wo_d = nc.dram_tensor("woP", [P, NC * D], F32, kind="ExternalInput")
    band_d = nc.dram_tensor("bandP", [P, NT * 3 * P], BF16, kind="ExternalInput")
    if with_biases:
        # rows: bq, bk, bv, bo
        bias_d = nc.dram_tensor("biases", [4, D], F32, kind="ExternalInput")
    out1_d = nc.dram_tensor("out1", [S, D], F32, kind="ExternalOutput")
    num_d = nc.dram_tensor("num", [S, D], F32, kind="ExternalOutput")

    with tile.TileContext(nc) as tc:
        with (
            tc.tile_pool(name="consts", bufs=1) as consts,
            tc.tile_pool(name="weights", bufs=1) as wpool,
            tc.tile_pool(name="ekk_pool", bufs=1) as ekkpool,
            tc.tile_pool(name="kv_in", bufs=3) as kvpool,
            tc.tile_pool(name="q_in", bufs=3) as qpool,
            tc.tile_pool(name="work", bufs=3) as work,
            tc.tile_pool(name="outs", bufs=3) as outs,
            tc.tile_pool(name="psum", bufs=1, space="PSUM") as psum,
        ):
            # ---- weights: wk/wv first so phase B starts ASAP ----
            w_sb = {}

            def load_w(name, d, dt=F32R):
                t = wpool.tile([P, NC, D], dt, tag=f"w_{name}", name=f"w_{name}")
                nc.sync.dma_start(
                    t, d[:, :].bitcast(dt).rearrange("p (c n) -> p c n", c=NC)
                )
                w_sb[name] = t

            def load_w_split(name, d, eng, pieces, dt=F32R):
                t = wpool.tile([P, NC, D], dt, tag=f"w_{name}", name=f"w_{name}")
                src = d[:, :].bitcast(dt).rearrange("p (c n) -> p c n", c=NC)
                step = NC // pieces
                for c0 in range(0, NC, step):
                    eng.dma_start(
                        t[:, c0 : c0 + step, :], src[:, c0 : c0 + step, :]
                    )
                w_sb[name] = t

            wk_t = wpool.tile([P, NC, D], IN_MM_DT, tag="w_wk", name="w_wk")
            wk_src = wk_d[:, :].bitcast(IN_MM_DT).rearrange("p (c n) -> p c n", c=NC)
            for n0 in range(0, D, P):  # chunk 0 in 64KB quarters, first
                nc.sync.dma_start(wk_t[:, 0, n0 : n0 + P], wk_src[:, 0, n0 : n0 + P])
            for c in range(1, NC):
                nc.sync.dma_start(wk_t[:, c, :], wk_src[:, c, :])
            w_sb["wk"] = wk_t
            load_w_split("wv", wv_d, nc.gpsimd, 4, IN_MM_DT)

            # ---- constants ----
            identity_f32 = consts.tile([P, P], F32)
            make_identity(nc, identity_f32)
            identity = consts.tile([P, P], F32R)
            nc.vector.tensor_copy(identity, identity_f32)
            ones_f32 = consts.tile([P, 1], F32)
            nc.gpsimd.memset(ones_f32, 1.0)
            ones_col = consts.tile([P, 1], F32R)  # lhsT for column sums
            nc.vector.tensor_copy(ones_col, ones_f32)
            ones_row = consts.tile([1, P], F32R)  # lhsT for partition bcast
            nc.vector.tensor_copy(ones_row, ones_f32[0:1, 0:1].broadcast_to([1, P]))

            if with_biases:
                bias_sb = consts.tile([4, D], F32R)
                nc.sync.dma_start(bias_sb, bias_d[:, :].bitcast(F32R))


            # ---- ekk: per seq-tile j, [ekv | ek] along free dim ----
            ekk = ekkpool.tile([P, NT, 2 * D], F32R)

            # kv input tiles: emit all DMAs up front so the rings stay fed
            kv_tiles = []
            for j in range(NT):
                ksrc = kP_d[j].bitcast(IN_MM_DT).rearrange("p (c t) -> p c t", c=NC)
                vsrc = vP_d[j].bitcast(IN_MM_DT).rearrange("p (c t) -> p c t", c=NC)
                kT_t = kvpool.tile([P, NC, P], IN_MM_DT, tag="kT_t", bufs=4, name="kT_t")
                vT_t = kvpool.tile([P, NC, P], IN_MM_DT, tag="vT_t", bufs=4, name="vT_t")
                step = 1 if j < 2 else 2
                for c0 in range(0, NC, step):
                    nc.sync.dma_start(
                        kT_t[:, c0 : c0 + step, :], ksrc[:, c0 : c0 + step, :]
                    )
                    nc.gpsimd.dma_start(
                        vT_t[:, c0 : c0 + step, :], vsrc[:, c0 : c0 + step, :]
                    )
                kv_tiles.append((kT_t, vT_t))

            ewm1 = wpool.tile([P, NT, 3 * P], F32R, tag="ewm1", name="ewm1")

            def proj_psum(xT_tile, wname, bias_row):
                """psum [128, 512] = (x @ W.T + b) for one 128-seq tile."""
                ps = psum.tile([P, D], F32, tag="ps", bufs=3, name="proj_ps")
                for c in range(NC):
                    nc.tensor.matmul(
                        ps,
                        xT_tile[:, c, :],
                        w_sb[wname][:, c, :],
                        start=(c == 0),
                        stop=(c == NC - 1 and bias_row is None),
                    )
                if bias_row is not None:
                    nc.tensor.matmul(
                        ps,
                        ones_row[:, :],
                        bias_sb[bias_row : bias_row + 1, :],
                        start=False,
                        stop=True,
                    )
                return ps

            # ---- phase B: ek / ekv + column sums ----
            csum_acc = wpool.tile([P, 2 * D], F32, tag="csum_acc", name="csum_acc")
            for j in range(NT):
                kT_t, vT_t = kv_tiles[j]

                if j == 2:
                    load_w("wq", wq_d, IN_MM_DT)
                    load_w("wo", wo_d)
                if j >= 9 and j <= 12:  # band chunks at j=9..12
                    ci = j - 9
                    bchunk = qpool.tile(
                        [P, 4, 3 * P], BF16, tag="bchunk", bufs=2, name="bchunk"
                    )
                    nc.gpsimd.dma_start(
                        bchunk,
                        band_d[:, ts(ci, 4 * 3 * P)].rearrange(
                            "p (i f) -> p i f", i=4
                        ),
                    )
                    ew_view = ewm1[:, ts(ci, 4), :].rearrange("p i f -> p (i f)")
                    bc_flat = bchunk.rearrange("p i f -> p (i f)")
                    nc.scalar.activation(ew_view, bc_flat, EXP)
                    nc.vector.tensor_scalar_add(
                        ew_view, ew_view.bitcast(F32), -1.0
                    )

                keyp_ps = proj_psum(kT_t, "wk", 1 if with_biases else None)
                ek_view = ekk[:, j, D : 2 * D]
                nc.scalar.activation(ek_view, keyp_ps, EXP)

                val_ps = proj_psum(vT_t, "wv", 2 if with_biases else None)
                nc.vector.tensor_mul(ekk[:, j, 0:D], ek_view.bitcast(F32), val_ps)

                # column-sum accumulation chain on DVE
                if j == 0:
                    nc.vector.tensor_copy(csum_acc, ekk[:, 0, :].bitcast(F32))
                else:
                    nc.vector.tensor_add(
                        csum_acc, csum_acc, ekk[:, j, :].bitcast(F32)
                    )

            # partition-reduce on the PE (ones^T @ acc), evict, and let each
            # band psum group add it back via a K=1 broadcast matmul
            csum_accr = wpool.tile([P, 2 * D], F32R, tag="csum_accr", name="csum_accr")
            nc.vector.tensor_copy(csum_accr, csum_acc)
            csum_sb = consts.tile([1, 2 * D], F32R)
            for h in range(2):
                cs_ps = psum.tile([1, D], F32, tag="xT", bufs=2, name=f"cs{h}")
                nc.tensor.matmul(
                    cs_ps, ones_col[:, :], csum_accr[:, h * D : (h + 1) * D],
                    start=True, stop=True,
                )
                nc.vector.tensor_copy(csum_sb[:, h * D : (h + 1) * D], cs_ps)

            # ---- phase C: per output row tile, software-pipelined so the
            # PE never waits on the DVE epilogue chain: the transpose +
            # output projection of tile i-1 are emitted after the band
            # matmuls of tile i ----
            def tail_stage(x_sb, i):
                xT_psum = psum.tile([P, NC, P], F32R, tag="xT", bufs=2, name="xT_ps")
                for c in range(NC):
                    nc.tensor.transpose(xT_psum[:, c, :], x_sb[:, ts(c, P)], identity)
                xT_sb = work.tile([P, NC, P], F32R, tag="xT_sb")
                nc.vector.tensor_copy(xT_sb, xT_psum)

                out_ps = proj_psum(xT_sb, "wo", 3 if with_biases else None)
                out_sb = outs.tile([P, D], F32, tag="out_sb")
                nc.vector.tensor_copy(out_sb, out_ps)
                if i >= NT - 2:
                    nc.gpsimd.dma_start(out1_d[ts(i, P), 0:256], out_sb[:, 0:256])
                    nc.gpsimd.dma_start(out1_d[ts(i, P), 256:512], out_sb[:, 256:512])
                else:
                    nc.gpsimd.dma_start(out1_d[ts(i, P), :], out_sb)

            pending = None
            for i in range(NT):
                qT_t = qpool.tile([P, NC, P], IN_MM_DT, tag="qT_t", bufs=4, name="qT_t")
                qsrc = qP_d[i].bitcast(IN_MM_DT).rearrange("p (c t) -> p c t", c=NC)
                nc.sync.dma_start(qT_t[:, 0:2, :], qsrc[:, 0:2, :])
                nc.sync.dma_start(qT_t[:, 2:4, :], qsrc[:, 2:4, :])
                query_ps = proj_psum(qT_t, "wq", 0 if with_biases else None)
                q_sb = work.tile([P, D], F32, tag="q_sb")
                nc.scalar.activation(q_sb, query_ps, SIGMOID)

                halves = {}
                for h in (1, 0):  # den first: rcp overlaps the num matmuls
                    ps = psum.tile([P, D], F32, tag="bps", bufs=3, name=f"bps{h}")
                    blocks = _band_blocks(i)
                    for bi, (jl, j) in enumerate(blocks):
                        nc.tensor.matmul(
                            ps,
                            ewm1[:, i, ts(jl, P)],
                            ekk[:, j, h * D : (h + 1) * D],
                            start=(bi == 0),
                            stop=False,
                        )
                    nc.tensor.matmul(
                        ps,
                        ones_row[:, :],
                        csum_sb[:, h * D : (h + 1) * D],
                        start=False,
                        stop=True,
                    )
                    halves[h] = ps
                num_ps, den_ps = halves[0], halves[1]

                rcp_sb = work.tile([P, D], F32, tag="rcp", bufs=2)
                nc.vector.reciprocal_approx_fast(out=rcp_sb, in_=den_ps)
                num_sb = outs.tile([P, D], F32, tag="num_sb")
                nc.vector.tensor_mul(num_sb, q_sb, num_ps)
                if i >= NT - 2:
                    nc.sync.dma_start(num_d[ts(i, P), 0:256], num_sb[:, 0:256])
                    nc.sync.dma_start(num_d[ts(i, P), 256:512], num_sb[:, 256:512])
                else:
                    nc.sync.dma_start(num_d[ts(i, P), :], num_sb)
                x_sb = work.tile([P, D], F32R, tag="x_sb")
                nc.vector.tensor_mul(x_sb, num_sb, rcp_sb)  # f32r out: cheap transpose

                if pending is not None:
                    tail_stage(*pending)
                pending = (x_sb, i)
            tail_stage(*pending)

    nc.finalize()
    return nc


def _pack_band(w_bias, local_mask):
    """[128, NT*384]: pack[t', i*384 + jl*128 + s'] =
    (w_bias*mask)[i*128+s', (i-1+jl)*128+t']  (transposed band blocks)."""
    wbm = np.asarray(w_bias, np.float32) * np.asarray(local_mask, np.float32)
    pack = np.zeros((NT, P, 3 * P), np.float32)
    for i in range(NT):
        for jl, j in _band_blocks(i):
            blk = wbm[i * P : (i + 1) * P, j * P : (j + 1) * P]
            pack[i, :, jl * P : (jl + 1) * P] = blk.T
    # anything |i-j| >= 2 tiles must be zero for the decomposition to hold
    for i in range(NT):
        lo = max(0, (i - 1) * P)
        hi = min(S, (i + 2) * P)
        row = wbm[i * P : (i + 1) * P]
        if row[:, :lo].any() or row[:, hi:].any():
            raise ValueError("w_bias*mask has support outside the 3-tile band")
    import ml_dtypes

    return np.ascontiguousarray(
        pack.transpose(1, 0, 2).reshape(P, NT * 3 * P).astype(ml_dtypes.bfloat16)
    )


def _pack_seq(x):
    """[S, D] -> [NT, 128, NC*128], pack[i, p, c*128+t] = x[i*128+t, c*128+p]."""
    out = np.ascontiguousarray(
        x.reshape(NT, P, NC, P).transpose(0, 3, 2, 1).reshape(NT, P, NC * P)
    )
    if USE_BF16_INPUTS:
        import ml_dtypes

        out = out.astype(ml_dtypes.bfloat16)
    return out


def _pack_w(w, bf16=False):
    """[D, D] -> [128, NC*512] with pack[p, c*512+n] = w[n, c*128+p]."""
    out = np.ascontiguousarray(
        np.asarray(w, np.float32)
        .T.reshape(NC, P, D)
        .transpose(1, 0, 2)
        .reshape(P, NC * D)
    )
    if bf16:
        import ml_dtypes

        out = out.astype(ml_dtypes.bfloat16)
    return out


_CACHE = {}


def _get_nc(with_biases):
    key = bool(with_biases)
    if key not in _CACHE:
        _CACHE[key] = build(key)
    return _CACHE[key]


def run(inputs, trace=False):
    q = np.asarray(inputs["q"], np.float32)
    k = np.asarray(inputs["k"], np.float32)
    v = np.asarray(inputs["v"], np.float32)
    B = q.shape[0]
    assert B == N_CORES and q.shape[1:] == (S, D)

    biases = np.stack(
        [
            np.asarray(inputs["Wq_b"], np.float32),
            np.asarray(inputs["Wk_b"], np.float32),
            np.asarray(inputs["Wv_b"], np.float32),
            np.asarray(inputs["out_b"], np.float32),
        ]
    )
    with_biases = bool(np.any(biases))

    shared = {
        "wqP": _pack_w(inputs["Wq_w"], USE_BF16_INPUTS),
        "wkP": _pack_w(inputs["Wk_w"], USE_BF16_INPUTS),
        "wvP": _pack_w(inputs["Wv_w"], USE_BF16_INPUTS),
        "woP": _pack_w(inputs["out_w"]),
        "bandP": _pack_band(inputs["w_bias"], inputs["local_mask"]),
    }
    if with_biases:
        shared["biases"] = biases

    in_maps = []
    for b in range(B):
        m = dict(shared)
        m["qP"] = _pack_seq(q[b])
        m["kP"] = _pack_seq(k[b])
        m["vP"] = _pack_seq(v[b])
        in_maps.append(m)

    nc = _get_nc(with_biases)
    res = run_bass_kernel_spmd(
        nc, in_maps, core_ids=list(range(N_CORES)), trace=trace
    )
    out1 = np.stack([res.results[b]["out1"] for b in range(B)])
    num = np.stack([res.results[b]["num"] for b in range(B)])
    return (out1, num), res


def kernel(**inputs):
    (out1, num), _ = run(inputs, trace=False)
    return (out1, num)
